# revision 1
# baseline (speedup 1.0000x reference)
"""CrossCosineEmbeddingLoss kernel for 8 trn2 NeuronCores.

loss = mean over all (i,j) of: 1 - cos(x_i, y_j) if i==j else relu(cos(x_i, y_j))

Identity:  total = sum_ij relu(sim_ij) + sum_i (1 - sim_ii - relu(sim_ii))
Sharding: rows of x across 8 cores (1024 rows each); y replicated, passed
row-major sliced (yd, for row norms / diagonal) and column-major (yt, the
matmul stationary operand) — pure layout copies, still fp32.

Per-core pipeline (v12): no dtype casts, no DMA transposes, no SWDGE.
  - yT fp32 chunks load straight into SBUF; matmuls use them as f32r
  - 1/||y_j|| computed distributed: each core does sumsq+rsqrt of its own
    1024 rows (from yd) and a 4KB AllGather shares all 8 slices; applied
    to the per-block row sums at the end (relu commutes with positive
    scaling), so the collective is fully latency-tolerant
  - x: sumsq -> 1/||x|| -> scale -> 8 PE transposes -> f32r xhatT
  - main: 64 j-tiles: 2 f32r matmuls -> [128,1024] fp32 PSUM -> fused
    relu+accum split across ACT and DVE
  - final: R * rny, reduce; diagonal correction from yd fp32 tiles
Host combines [128,2] partials from each core.
"""

import numpy as np

import concourse.bacc as bacc
import concourse.bass as bass
import concourse.tile as tile
from concourse import mybir
from concourse.bass_utils import run_bass_kernel_spmd
from concourse.masks import make_identity

N, D = 8192, 128
NCORES = 8
SH = N // NCORES          # 1024 rows of x per core
TX = SH // 128            # 8 x-tiles per core
TY = N // 128             # 64 y-tiles
YG = 8                    # y chunks (8 tiles each)

f32 = mybir.dt.float32
f32r = mybir.dt.float32r
AF = mybir.ActivationFunctionType
ALU = mybir.AluOpType
AX = mybir.AxisListType

ACT_TILES = 35              # of 64 main tiles handled by ACT (rest DVE)


def _reduce_kind(t):
    # first tiles go to ACT (DVE still finishing prep work during fill),
    # remainder alternates to hit ACT_TILES total
    lead = 2 * ACT_TILES - TY
    if t < lead:
        return "act"
    return "dve" if (t - lead) % 2 == 0 else "act"


_CACHE = {}


def _build():
    if "nc" in _CACHE:
        return _CACHE["nc"]
    nc = bacc.Bacc("TRN2", target_bir_lowering=False, debug=False,
                   num_devices=NCORES)
    xs_d = nc.dram_tensor("xs", [SH, D], f32, kind="ExternalInput")
    yd_d = nc.dram_tensor("yd", [SH, D], f32, kind="ExternalInput")
    yt_d = nc.dram_tensor("yt", [D, N], f32r, kind="ExternalInput")
    out_d = nc.dram_tensor("out", [128, 2], f32, kind="ExternalOutput")
    rl_d = nc.dram_tensor("rny_loc", [TX, 128], f32, kind="Internal")
    ra_d = nc.dram_tensor("rny_all", [TY, 128], f32, kind="Internal")

    with tile.TileContext(nc) as tc:
        with (
            tc.tile_pool(name="singles", bufs=1) as singles,
            tc.tile_pool(name="scrD", bufs=2) as scrD,
        ):
            yT32 = singles.tile([128, TY, 128], f32r)   # [d, jt, j]
            ydnat = singles.tile([128, TX, 128], f32)   # local y rows
            xnat = singles.tile([128, TX, 128], f32)    # [i%128, it, d]
            xhat = singles.tile([128, TX, 128], f32)
            xhatT = singles.tile([128, TX, 128], f32r)  # [d, it, i]
            ident = singles.tile([128, 128], f32)
            nx2 = singles.tile([128, TX], f32)
            rnx = singles.tile([128, TX], f32)
            nyd2 = singles.tile([128, TX], f32)
            rnyd = singles.tile([128, TX], f32)
            rnydT = singles.tile([128, 128], f32)   # rows 0:TX used
            rnyaT = singles.tile([64, 128], f32)    # gathered, t-major
            rny = singles.tile([128, TY], f32)
            t1x = singles.tile([128, TX], f32)
            R = singles.tile([128, TY], f32)
            d2 = singles.tile([128, TX], f32)
            sim_d = singles.tile([128, TX], f32)
            relu_d = singles.tile([128, TX], f32)
            outsb = singles.tile([128, 2], f32)
            warm = singles.tile([128, 8], f32)

            # preload the sqrt activation table set early (overlaps DMA)
            nc.vector.memset(warm[:], 1.0)
            nc.scalar.sqrt(warm[:], warm[:])
            make_identity(nc, ident[:])

            # ---- small loads first: x shard + local y rows
            nc.sync.dma_start(
                out=xnat[:], in_=xs_d[:].rearrange("(t p) d -> p t d", p=128))
            nc.sync.dma_start(
                out=ydnat[:], in_=yd_d[:].rearrange("(t p) d -> p t d", p=128))

            # ---- yT in geometric chunks: a small first chunk unblocks
            # the main loop early; few queues keep the round-robin share
            # of the small x/yd loads large
            ycut = (0, 4, 12, 28, TY)
            for g in range(len(ycut) - 1):
                gs = slice(ycut[g], ycut[g + 1])
                nc.sync.dma_start(
                    out=yT32[:, gs, :],
                    in_=yt_d[:, 128 * ycut[g]:128 * ycut[g + 1]]
                    .rearrange("p (t j) -> p t j", j=128))

            # ---- x norms + scale + transpose, pipelined per tile so the
            # first xhatT half is ready well before the last sumsq
            with tc.tile_pool(name="tpsum", bufs=1, space="PSUM") as tpsum:
                ptx = tpsum.tile([128, 1024], f32, tag="tp")
                xt_flat = xhatT[:].rearrange("p a b -> p (a b)")
                for t in range(TX):
                    nc.vector.scalar_tensor_tensor(
                        out=scrD.tile([128, 128], f32, tag='sd', name='sd')[:],
                        in0=xnat[:, t, :], scalar=1.0, in1=xnat[:, t, :],
                        op0=ALU.mult, op1=ALU.mult,
                        accum_out=nx2[:, t:t + 1])
                    nc.vector.reciprocal(t1x[:, t:t + 1], nx2[:, t:t + 1])
                    nc.scalar.sqrt(rnx[:, t:t + 1], t1x[:, t:t + 1])
                    nc.vector.tensor_scalar(
                        out=xhat[:, t, :], in0=xnat[:, t, :],
                        scalar1=rnx[:, t:t + 1], scalar2=None,
                        op0=ALU.mult, op1=ALU.bypass)
                    nc.tensor.transpose(ptx[:, 128 * t:128 * (t + 1)],
                                        xhat[:, t, :], ident[:])
                    if t == 3:
                        nc.vector.tensor_copy(out=xt_flat[:, 0:512],
                                              in_=ptx[:, 0:512])
                nc.vector.tensor_copy(out=xt_flat[:, 512:1024],
                                      in_=ptx[:, 512:1024])

                # ---- local y norms -> rnyd [128, TX]; transpose + gather
                for t in range(TX):
                    nc.vector.scalar_tensor_tensor(
                        out=scrD.tile([128, 128], f32, tag='sq', name='sq')[:],
                        in0=ydnat[:, t, :], scalar=1.0, in1=ydnat[:, t, :],
                        op0=ALU.mult, op1=ALU.mult,
                        accum_out=nyd2[:, t:t + 1])
                nc.vector.reciprocal(t1x[:], nyd2[:])
                nc.scalar.sqrt(rnyd[:], t1x[:])  # 1/||y_i|| local rows
                ptr = tpsum.tile([128, 128], f32, tag="tq")
                nc.tensor.transpose(ptr[0:TX, :], rnyd[:], ident[:])
                nc.vector.tensor_copy(out=rnydT[0:TX, :], in_=ptr[0:TX, :])
            nc.sync.dma_start(out=rl_d[:], in_=rnydT[0:TX, :])
            nc.gpsimd.collective_compute(
                kind="AllGather", op=ALU.bypass,
                replica_groups=[list(range(NCORES))],
                ins=[rl_d[:]], outs=[ra_d[:]])

            # ---- diagonal: raw dots from fp32 tiles, then scale
            for t in range(TX):
                nc.vector.scalar_tensor_tensor(
                    out=scrD.tile([128, 128], f32, tag='dg', name='dg')[:],
                    in0=xnat[:, t, :], scalar=1.0, in1=ydnat[:, t, :],
                    op0=ALU.mult, op1=ALU.mult, accum_out=d2[:, t:t + 1])
            nc.vector.tensor_mul(t1x[:], d2[:], rnx[:])
            nc.vector.tensor_mul(sim_d[:], t1x[:], rnyd[:])
            nc.scalar.activation(relu_d[:], sim_d[:], AF.Relu)
            nc.vector.scalar_tensor_tensor(
                out=scrD.tile([128, TX], f32, tag='df', name='df')[:],
                in0=sim_d[:], scalar=1.0, in1=relu_d[:],
                op0=ALU.mult, op1=ALU.add, accum_out=outsb[:, 1:2])

            # ---- main: per j-block f32r matmuls + fused relu-accumulate
            # ACT accumulators land in a PSUM bank (ScE->PSUM is faster
            # than ScE->SBUF); DVE accumulators stay in SBUF R
            nc.vector.memset(R[:], 0.0)
            with tc.tile_pool(name="mpsum", bufs=3, space="PSUM") as mpsum:
                with tc.tile_pool(name="rpsum", bufs=1, space="PSUM") as rp:
                    Rp = rp.tile([128, TY], f32, tag="racc")
                    nc.vector.memset(Rp[:], 0.0)
                    rhs = xhatT[:].rearrange("p a b -> p (a b)")
                    for t in range(TY):
                        ps = mpsum.tile([128, 1024], f32, tag="mp")
                        lhsT = yT32[:, t, :]
                        nc.tensor.matmul(ps[:, 0:512], lhsT, rhs[:, 0:512])
                        nc.tensor.matmul(ps[:, 512:1024], lhsT,
                                         rhs[:, 512:1024])
                        if _reduce_kind(t) == "act":
                            nc.scalar.activation(
                                ps[:], ps[:], AF.Relu,
                                accum_out=Rp[:, t:t + 1])
                        else:
                            nc.vector.tensor_scalar(
                                out=ps[:], in0=ps[:], scalar1=0.0,
                                scalar2=None, op0=ALU.max, op1=ALU.add,
                                accum_out=R[:, t:t + 1])
                    nc.vector.tensor_add(R[:], R[:], Rp[:])

            # ---- gathered rny: load t-major, PE transpose to [128, TY]
            nc.sync.dma_start(out=rnyaT[:], in_=ra_d[:])
            with tc.tile_pool(name="gpsum", bufs=1, space="PSUM") as gpsum:
                ptg = gpsum.tile([128, 64], f32, tag="tg")
                nc.tensor.transpose(ptg[:], rnyaT[:], ident[0:64, 0:64])
                nc.vector.tensor_copy(out=rny[:], in_=ptg[:])

            # ---- final: scale per-block sums by 1/||y_j|| and total,
            # fused into one DVE op (tensor_reduce is the slow 1x path)
            nc.vector.scalar_tensor_tensor(
                out=scrD.tile([128, TY], f32, tag='fs', name='fs')[:],
                in0=R[:], scalar=1.0, in1=rny[:],
                op0=ALU.mult, op1=ALU.mult, accum_out=outsb[:, 0:1])
            nc.sync.dma_start(out=out_d[:], in_=outsb[:])

    nc.compile()
    _CACHE["nc"] = nc
    return nc


def _in_maps(x, y):
    yt = np.ascontiguousarray(y.T)
    maps = []
    for c in range(NCORES):
        sl = slice(SH * c, SH * (c + 1))
        maps.append({"xs": np.ascontiguousarray(x[sl]),
                     "yd": np.ascontiguousarray(y[sl]),
                     "yt": yt})
    return maps


def _combine(results):
    total = 0.0
    for c in range(NCORES):
        o = results[c]["out"].astype(np.float64)
        total += o[:, 0].sum() - o[:, 1].sum() + SH
    return np.float32(total / (float(N) * float(N)))


def _run(x, y, trace=False):
    nc = _build()
    res = run_bass_kernel_spmd(nc, _in_maps(x, y), list(range(NCORES)),
                               trace=trace)
    return _combine(res.results), res


def kernel(x, y):
    x = np.asarray(x, dtype=np.float32)
    y = np.asarray(y, dtype=np.float32)
    loss, _ = _run(x, y, trace=False)
    return loss



# revision 7
# speedup vs baseline: 1.0803x; 1.0803x over previous
"""CrossCosineEmbeddingLoss kernel for 8 trn2 NeuronCores.

loss = mean over all (i,j) of: 1 - cos(x_i, y_j) if i==j else relu(cos(x_i, y_j))

Identity:  total = sum_ij relu(sim_ij) + sum_i (1 - sim_ii - relu(sim_ii))
Sharding: rows of x across 8 cores (1024 rows each); y replicated, passed
row-major sliced (yd, for row norms / diagonal) and column-major (yt, the
matmul stationary operand) — pure layout copies, still fp32.

Per-core pipeline (v13): ACT+DVE do nothing but the PSUM relu+accumulate
main loop (the 2-engine floor); ALL SBUF-side prep moves to the otherwise
idle Pool/GpSimd engine (x sumsq+scale, local-y sumsq, diagonal dots).
  - Rsqrt activation replaces reciprocal+sqrt (one table set with Relu)
  - x / yd DMAs split across multiple queues; yT in 5 geometric chunks
  - 1/||y_j|| distributed: local rows' rsqrt + 4KB AllGather, issued as
    early as possible (collective_compute placed after all Pool work on
    the queue); applied to per-block row sums at the end
  - main: 64 j-tiles: 2 f32r matmuls -> [128,1024] fp32 PSUM -> fused
    relu+accum alternating ACT (even tiles, PSUM accum) / DVE (odd, SBUF)
Host combines [128,2] partials from each core.
"""

import numpy as np

import concourse.bacc as bacc
import concourse.bass as bass
import concourse.tile as tile
from concourse import mybir
from concourse.bass_utils import run_bass_kernel_spmd
from concourse.masks import make_identity

N, D = 8192, 128
NCORES = 8
SH = N // NCORES          # 1024 rows of x per core
TX = SH // 128            # 8 x-tiles per core
TY = N // 128             # 64 y-tiles

f32 = mybir.dt.float32
f32r = mybir.dt.float32r
AF = mybir.ActivationFunctionType
ALU = mybir.AluOpType

ACT_TILES = 35              # of 64 main tiles handled by ACT (rest DVE)


def _reduce_kind(t):
    lead = 2 * ACT_TILES - TY
    if t < lead:
        return "act"
    return "dve" if (t - lead) % 2 == 0 else "act"


_CACHE = {}


def _build():
    if "nc" in _CACHE:
        return _CACHE["nc"]
    nc = bacc.Bacc("TRN2", target_bir_lowering=False, debug=False,
                   num_devices=NCORES)
    xs_d = nc.dram_tensor("xs", [SH, D], f32, kind="ExternalInput")
    yd_d = nc.dram_tensor("yd", [SH, D], f32, kind="ExternalInput")
    yt_d = nc.dram_tensor("yt", [D, N], f32r, kind="ExternalInput")
    out_d = nc.dram_tensor("out", [128, 2], f32, kind="ExternalOutput")
    rl_d = nc.dram_tensor("rny_loc", [TX, 128], f32, kind="Internal")
    ra_d = nc.dram_tensor("rny_all", [TY, 128], f32, kind="Internal")

    with tile.TileContext(nc) as tc:
        with (
            tc.tile_pool(name="singles", bufs=1) as singles,
            tc.tile_pool(name="scrD", bufs=2) as scrD,
            tc.tile_pool(name="scrP", bufs=2) as scrP,
        ):
            yT32 = singles.tile([128, TY, 128], f32r)   # [d, jt, j]
            ydnat = singles.tile([128, TX, 128], f32)   # local y rows
            xnat = singles.tile([128, TX, 128], f32)    # [i%128, it, d]
            xhat = singles.tile([128, TX, 128], f32)
            xhatT = singles.tile([128, TX, 128], f32r)  # [d, it, i]
            ident = singles.tile([128, 128], f32)
            nx2 = singles.tile([128, TX], f32)
            rnx = singles.tile([128, TX], f32)
            nyd2 = singles.tile([128, TX], f32)
            rnyd = singles.tile([128, TX], f32)
            rnydT = singles.tile([128, 128], f32)   # rows 0:TX used
            rnyaT = singles.tile([64, 128], f32)    # gathered, t-major
            rny = singles.tile([128, TY], f32)
            R = singles.tile([128, TY], f32)
            d2 = singles.tile([128, TX], f32)
            t1x = singles.tile([128, TX], f32)
            sim_d = singles.tile([128, TX], f32)
            relu_d = singles.tile([128, TX], f32)
            outsb = singles.tile([128, 2], f32)
            warm = singles.tile([128, 8], f32)

            # preload the sqrt+relu activation table set early
            nc.gpsimd.memset(warm[:], 1.0)
            nc.scalar.sqrt(warm[:], warm[:])
            make_identity(nc, ident[:])
            nc.gpsimd.memset(outsb[:], 0.0)
            nc.gpsimd.memset(R[:], 0.0)

            # ---- DMA: x shard split across queues, then first yT chunk,
            # then local y rows, then remaining yT chunks by priority
            h = TX // 2
            nc.sync.dma_start(
                out=xnat[:, 0:h, :],
                in_=xs_d[0:SH // 2].rearrange("(t p) d -> p t d", p=128))
            nc.sync.dma_start(
                out=xnat[:, h:TX, :],
                in_=xs_d[SH // 2:SH].rearrange("(t p) d -> p t d", p=128))
            ycut = (0, 4, 12, 24, 40, TY)
            for g in range(len(ycut) - 1):
                gs = slice(ycut[g], ycut[g + 1])
                nc.sync.dma_start(
                    out=yT32[:, gs, :],
                    in_=yt_d[:, 128 * ycut[g]:128 * ycut[g + 1]]
                    .rearrange("p (t j) -> p t j", j=128))
                if g == 0:
                    nc.sync.dma_start(
                        out=ydnat[:, 0:h, :],
                        in_=yd_d[0:SH // 2].rearrange("(t p) d -> p t d",
                                                      p=128))
                    nc.sync.dma_start(
                        out=ydnat[:, h:TX, :],
                        in_=yd_d[SH // 2:SH].rearrange("(t p) d -> p t d",
                                                       p=128))

            # ---- x prep: Pool does sumsq + scale, ACT does rsqrt,
            # PE transposes, DVE copies PSUM->SBUF
            with tc.tile_pool(name="tpsum", bufs=1, space="PSUM") as tpsum:
                ptx = tpsum.tile([128, 1024], f32, tag="tp")
                xt_flat = xhatT[:].rearrange("p a b -> p (a b)")
                for t in range(TX):
                    nc.vector.scalar_tensor_tensor(
                        out=scrD.tile([128, 128], f32, tag='sd', name='sd')[:],
                        in0=xnat[:, t, :], scalar=1.0, in1=xnat[:, t, :],
                        op0=ALU.mult, op1=ALU.mult,
                        accum_out=nx2[:, t:t + 1])
                    nc.vector.reciprocal(t1x[:, t:t + 1], nx2[:, t:t + 1])
                    nc.scalar.sqrt(rnx[:, t:t + 1], t1x[:, t:t + 1])
                    nc.vector.tensor_scalar(
                        out=xhat[:, t, :], in0=xnat[:, t, :],
                        scalar1=rnx[:, t:t + 1], scalar2=None,
                        op0=ALU.mult, op1=ALU.bypass)
                    nc.tensor.transpose(ptx[:, 128 * t:128 * (t + 1)],
                                        xhat[:, t, :], ident[:])
                    if t == 3:
                        nc.vector.tensor_copy(out=xt_flat[:, 0:512],
                                              in_=ptx[:, 0:512])
                nc.vector.tensor_copy(out=xt_flat[:, 512:1024],
                                      in_=ptx[:, 512:1024])

                # ---- local y norms on Pool -> ACT rsqrt -> transpose
                for t in range(TX):
                    nc.vector.scalar_tensor_tensor(
                        out=scrD.tile([128, 128], f32, tag='sq', name='sq')[:],
                        in0=ydnat[:, t, :], scalar=1.0, in1=ydnat[:, t, :],
                        op0=ALU.mult, op1=ALU.mult,
                        accum_out=nyd2[:, t:t + 1])
                nc.vector.reciprocal(nyd2[:], nyd2[:])
                nc.scalar.sqrt(rnyd[:], nyd2[:])
                ptr = tpsum.tile([128, 128], f32, tag="tq")
                nc.tensor.transpose(ptr[0:TX, :], rnyd[:], ident[:])
                nc.vector.tensor_copy(out=rnydT[0:TX, :], in_=ptr[0:TX, :])
            nc.sync.dma_start(out=rl_d[:], in_=rnydT[0:TX, :])

            # ---- diagonal dots on Pool (raw x.y per local row)
            for t in range(TX):
                nc.vector.scalar_tensor_tensor(
                    out=scrD.tile([128, 128], f32, tag='dg', name='dg')[:],
                    in0=xnat[:, t, :], scalar=1.0, in1=ydnat[:, t, :],
                    op0=ALU.mult, op1=ALU.mult, accum_out=d2[:, t:t + 1])
            nc.gpsimd.tensor_mul(t1x[:], d2[:], rnx[:])
            nc.gpsimd.tensor_mul(sim_d[:], t1x[:], rnyd[:])

            # collective must be LAST on the Pool queue (it blocks the
            # sequencer until the AllGather lands)
            nc.gpsimd.collective_compute(
                kind="AllGather", op=ALU.bypass,
                replica_groups=[list(range(NCORES))],
                ins=[rl_d[:]], outs=[ra_d[:]])

            # ---- main: per j-block f32r matmuls + fused relu-accumulate
            # ACT takes even tiles (accum into PSUM Rp), DVE odd (SBUF R)
            with tc.tile_pool(name="mpsum", bufs=3, space="PSUM") as mpsum:
                with tc.tile_pool(name="rpsum", bufs=1, space="PSUM") as rp:
                    Rp = rp.tile([128, TY], f32, tag="racc")
                    nc.vector.memset(Rp[:], 0.0)
                    rhs = xhatT[:].rearrange("p a b -> p (a b)")
                    for t in range(TY):
                        ps = mpsum.tile([128, 1024], f32, tag="mp")
                        lhsT = yT32[:, t, :]
                        nc.tensor.matmul(ps[:, 0:512], lhsT, rhs[:, 0:512])
                        nc.tensor.matmul(ps[:, 512:1024], lhsT,
                                         rhs[:, 512:1024])
                        if _reduce_kind(t) == "act":
                            nc.scalar.activation(
                                ps[:], ps[:], AF.Relu,
                                accum_out=Rp[:, t:t + 1])
                        else:
                            nc.vector.tensor_scalar(
                                out=ps[:], in0=ps[:], scalar1=0.0,
                                scalar2=None, op0=ALU.max, op1=ALU.add,
                                accum_out=R[:, t:t + 1])
                        if t == 8:
                            # diagonal correction, off the critical path:
                            # relu on ACT, sum on DVE
                            nc.scalar.activation(relu_d[:], sim_d[:],
                                                 AF.Relu)
                            nc.vector.scalar_tensor_tensor(
                                out=scrD.tile([128, TX], f32, tag='df',
                                              name='df')[:],
                                in0=sim_d[:], scalar=1.0, in1=relu_d[:],
                                op0=ALU.mult, op1=ALU.add,
                                accum_out=outsb[:, 1:2])
                    nc.vector.tensor_add(R[:], R[:], Rp[:])

            # ---- gathered rny: load t-major, PE transpose to [128, TY]
            nc.sync.dma_start(out=rnyaT[:], in_=ra_d[:])
            with tc.tile_pool(name="gpsum", bufs=1, space="PSUM") as gpsum:
                ptg = gpsum.tile([128, 64], f32, tag="tg")
                nc.tensor.transpose(ptg[:], rnyaT[:], ident[0:64, 0:64])
                nc.vector.tensor_copy(out=rny[:], in_=ptg[:])

            # ---- final: scale per-block sums by 1/||y_j|| and total
            nc.vector.scalar_tensor_tensor(
                out=scrD.tile([128, TY], f32, tag='fs', name='fs')[:],
                in0=R[:], scalar=1.0, in1=rny[:],
                op0=ALU.mult, op1=ALU.mult, accum_out=outsb[:, 0:1])
            nc.sync.dma_start(out=out_d[:], in_=outsb[:])

    nc.compile()
    _CACHE["nc"] = nc
    return nc


def _in_maps(x, y):
    yt = np.ascontiguousarray(y.T)
    maps = []
    for c in range(NCORES):
        sl = slice(SH * c, SH * (c + 1))
        maps.append({"xs": np.ascontiguousarray(x[sl]),
                     "yd": np.ascontiguousarray(y[sl]),
                     "yt": yt})
    return maps


def _combine(results):
    total = 0.0
    for c in range(NCORES):
        o = results[c]["out"].astype(np.float64)
        total += o[:, 0].sum() - o[:, 1].sum() + SH
    return np.float32(total / (float(N) * float(N)))


def _run(x, y, trace=False):
    nc = _build()
    res = run_bass_kernel_spmd(nc, _in_maps(x, y), list(range(NCORES)),
                               trace=trace)
    return _combine(res.results), res


def kernel(x, y):
    x = np.asarray(x, dtype=np.float32)
    y = np.asarray(y, dtype=np.float32)
    loss, _ = _run(x, y, trace=False)
    return loss
